# revision 1
# baseline (speedup 1.0000x reference)
"""Trainium2 Bass kernel for nn_Cross_attention_3 (sparse_attention).

Sharding: the (D, H*W) plane is unfolded into 9x9 patches; D=72 gives 8
patch-row blocks of 9 rows — exactly one per NeuronCore.  The only
cross-core dependency is the AdaptiveAvgPool over the patch axis
(bins of 128 patches straddle core boundaries); cores exchange 8-patch
half-block partial sums via a 3.4MB AllGather.

The two MLP linears have no nonlinearity between them, so they collapse
into a single 81x81 matrix; the conv bias rides along as an 82nd
contraction row whose rhs holds b[c].  The 1x1x1 conv is computed with
the patch data as the matmul's stationary operand, so its output lands
directly in (patch-element, channel) layout — the transpose the rest of
the pipeline needs comes for free.  Patches are packed in slot-halves
(slot 0 = patches 0..647, slot 1 = 648..1295) so pooling windows and
attention output runs stay contiguous.
"""

import os
import sys

import numpy as np

try:
    import ml_dtypes
except ImportError:
    ml_dtypes = None

try:
    import concourse.bacc as _  # noqa: F401
except ImportError:  # container default path
    sys.path.insert(0, "/opt/trn_rl_repo")

import concourse.bacc as bacc
import concourse.mybir as mybir
from concourse.bass_utils import run_bass_kernel_spmd
from concourse.tile import TileContext

P = 9
P2 = 81
C = 64
D = 72
H = W = 108
HW = H * W
NCORES = 8
LLOC = HW // P   # 1296 patches per core
LP = LLOC // 2   # 648 patch pairs per core (slot halves)

NLP = 24                     # chunk size in pairs, both passes
NCH_A = LP // NLP            # 27 chunks
RING_B = 216                 # pass-B ring, in pairs
NRING = LP // RING_B         # 3
SUB_B = 24                   # pass-B subchunk, in pairs
NHALF = LP // 8              # 81 half-blocks (8 patches) per slot

F32 = mybir.dt.float32
BF16 = mybir.dt.bfloat16

_cache = {}


def _build_nc():
    nc = bacc.Bacc(None, target_bir_lowering=False, debug=False)
    xp_d = nc.declare_dram_parameter("xp", [128, LP, P2], BF16, isOutput=False)
    yp_d = nc.declare_dram_parameter("yp", [128, LP, P2], BF16, isOutput=False)
    wi_d = nc.declare_dram_parameter("wi", [128, 128], BF16, isOutput=False)
    wf_d = nc.declare_dram_parameter("wf", [128, 128], BF16, isOutput=False)
    wm_d = nc.declare_dram_parameter("wm", [82, P2], BF16, isOutput=False)
    bi_d = nc.declare_dram_parameter("bi", [1, SUB_B * 128], BF16, isOutput=False)
    bf_d = nc.declare_dram_parameter("bf", [1, SUB_B * 128], BF16, isOutput=False)
    out_d = nc.declare_dram_parameter("out", [P2, C, 2 * LP], BF16, isOutput=True)

    # per-slot 8-patch half-block sums; gathered across cores
    gath_d = nc.dram_tensor("gath", [NCORES, P2, C, NHALF], F32,
                            addr_space="Shared")

    with nc.allow_low_precision("bf16 compute pipeline"), TileContext(nc) as tc:
        with (
            tc.tile_pool(name="const", bufs=1) as constp,
            tc.tile_pool(name="psconv", bufs=4, space="PSUM") as psconv,
            tc.tile_pool(name="psmlp", bufs=2, space="PSUM") as psmlp,
            tc.tile_pool(name="dram", bufs=1, space="DRAM") as dramp,
        ):
            wi_sb = constp.tile([128, 128], BF16, tag="wi")
            wf_sb = constp.tile([128, 128], BF16, tag="wf")
            wm_sb = constp.tile([82, P2], BF16, tag="wm")
            pooled = constp.tile([P2, C, P2], BF16, tag="pooled")
            h_dram = dramp.tile([P2, C, NHALF], F32)
            nc.sync.dma_start(out=wi_sb[:, :], in_=wi_d[:, :])
            nc.sync.dma_start(out=wf_sb[:, :], in_=wf_d[:, :])
            nc.sync.dma_start(out=wm_sb[:, :], in_=wm_d[:, :])

            def conv_mlp(stp, mlq, nlp, src_d, w_sb, b_d, lp0, dst, dst_lp0,
                         dst_nlp, act_ix):
                """conv+MLP+lrelu for NLP pairs starting at pair lp0 of src_d.
                dst: (81, dst_nlp, 128) bf16 tile, (lp, sc)-major, written at
                lp offset dst_lp0."""
                st = stp.tile([128, nlp, P2], BF16, tag="stage")
                nc.sync.dma_start(out=st[:, :, :], in_=src_d[:, lp0:lp0 + nlp, :])
                ms = mlq.tile([82, nlp, 128], BF16, tag="ms")
                nc.sync.dma_start(
                    out=ms[81:82, :, :].rearrange("p a b -> p (a b)"),
                    in_=b_d[:, 0:nlp * 128],
                )
                # conv: 4 pairs per PSUM bank, N=128 each; one contiguous
                # evict per bank into ms (lp-major); DVE 2/3, ACT 1/3
                for pb in range(nlp // 4):
                    ps = psconv.tile([P2, 512], F32, tag="psc")
                    for j in range(4):
                        nc.tensor.matmul(
                            ps[0:P2, 128 * j:128 * (j + 1)],
                            st[:, 4 * pb + j, :], w_sb[:, :],
                            start=True, stop=True,
                        )
                    d = ms[0:P2, 4 * pb:4 * pb + 4, :].rearrange("p a b -> p (a b)")
                    nc.vector.tensor_copy(d, ps[0:P2, 0:512])
                # MLP (+bias row) and LeakyReLU on ACT; 512 cols = 4 lp
                flat = ms[:, :, :].rearrange("p a b -> p (a b)")
                for q in range(nlp // 4):
                    mp = psmlp.tile([P2, 512], F32, tag="psm")
                    nc.tensor.matmul(
                        mp[0:P2, :], wm_sb[:, :], flat[:, 512 * q:512 * (q + 1)],
                        start=True, stop=True,
                    )
                    dq = dst[0:P2, dst_lp0 + 4 * q:dst_lp0 + 4 * q + 4,
                             :].rearrange("p a b -> p (a b)")
                    nc.scalar.activation(
                        dq, mp[0:P2, :],
                        mybir.ActivationFunctionType.Prelu, alpha=0.2,
                    )

            # ---------------- pass A: fea (y) + pooled ----------------------
            with (
                tc.tile_pool(name="stageA", bufs=3) as stagep,
                tc.tile_pool(name="mlpsA", bufs=2) as mlpp,
                tc.tile_pool(name="feaout", bufs=2) as feap,
                tc.tile_pool(name="pooltmp", bufs=2) as ptp,
                tc.tile_pool(name="hsb", bufs=1) as hsbp,
                tc.tile_pool(name="comb", bufs=2) as combp,
            ):
                # h_sb: 8-patch half-block sums, (81, 162, 64): dim1 =
                # slot*81 + h, dim2 = c
                h_sb = hsbp.tile([P2, 2 * NHALF, C], F32, tag="hsb")
                s_sb = hsbp.tile([P2, C, NHALF], F32, tag="ssb")
                for ch in range(NCH_A):
                    fea = feap.tile([P2, NLP, 128], BF16, tag="fea")
                    conv_mlp(stagep, mlpp, NLP, yp_d, wf_sb, bf_d, ch * NLP,
                             fea, 0, NLP, ch)
                    # pairwise tree over lp: 24 -> 12 -> 6 -> 3 half-sums
                    t1 = ptp.tile([P2, 12, 128], BF16, tag="t1")
                    f2 = fea[0:P2, :, :].rearrange("p (a two) b -> p a two b", two=2)
                    nc.vector.tensor_tensor(
                        t1[0:P2, :, :], f2[:, :, 0, :], f2[:, :, 1, :],
                        mybir.AluOpType.add,
                    )
                    t2 = ptp.tile([P2, 6, 128], BF16, tag="t2")
                    t1s = t1[0:P2, :, :].rearrange("p (a two) b -> p a two b", two=2)
                    nc.vector.tensor_tensor(
                        t2[0:P2, :, :], t1s[:, :, 0, :], t1s[:, :, 1, :],
                        mybir.AluOpType.add,
                    )
                    t3 = ptp.tile([P2, 3, 128], F32, tag="t3")
                    t2s = t2[0:P2, :, :].rearrange("p (a two) b -> p a two b", two=2)
                    nc.vector.tensor_tensor(
                        t3[0:P2, :, :], t2s[:, :, 0, :], t2s[:, :, 1, :],
                        mybir.AluOpType.add,
                    )
                    # scatter the 3 half-sums per slot into h_sb
                    for slot in range(2):
                        nc.vector.tensor_copy(
                            h_sb[0:P2, NHALF * slot + 3 * ch:
                                 NHALF * slot + 3 * ch + 3, :],
                            t3[0:P2, :, 64 * slot:64 * slot + 64],
                        )
                # merge halves into 81 16-patch blocks (S), c-major for the
                # gather: S[e, c, s]
                hv = h_sb[0:P2, :, :]
                # s in [0, 40): slot0 pairs (2s, 2s+1)
                e0 = hv[:, 0:80, :].rearrange("p (a two) c -> p a two c", two=2)
                nc.vector.tensor_tensor(
                    s_sb[0:P2, :, 0:40].rearrange("p c s -> p s c"),
                    e0[:, :, 0, :], e0[:, :, 1, :], mybir.AluOpType.add,
                )
                # s = 40: slot0 h=80 + slot1 h=0
                nc.vector.tensor_tensor(
                    s_sb[0:P2, :, 40:41].rearrange("p c s -> p s c"),
                    hv[:, 80:81, :], hv[:, 81:82, :], mybir.AluOpType.add,
                )
                # s in [41, 81): slot1 pairs (1+2t, 2+2t)
                e1 = hv[:, 82:162, :].rearrange("p (a two) c -> p a two c", two=2)
                nc.vector.tensor_tensor(
                    s_sb[0:P2, :, 41:81].rearrange("p c s -> p s c"),
                    e1[:, :, 0, :], e1[:, :, 1, :], mybir.AluOpType.add,
                )
                nc.gpsimd.dma_start(out=h_dram[:, :, :], in_=s_sb[:, :, :])
                nc.gpsimd.collective_compute(
                    "AllGather",
                    mybir.AluOpType.bypass,
                    replica_groups=[list(range(NCORES))],
                    ins=[h_dram[:, :, :]],
                    outs=[gath_d[:, :, :, :]],
                )
                # bins of 128 = 8 consecutive global 16-blocks (s_g = 81k + s)
                for cc in range(8):
                    tcb = combp.tile([P2, 8, NCORES * NHALF], F32, tag="tcb")
                    for k in range(NCORES):
                        nc.gpsimd.dma_start(
                            out=tcb[0:P2, :, NHALF * k:NHALF * (k + 1)],
                            in_=gath_d[k, :, 8 * cc:8 * (cc + 1), :],
                        )
                    pr = combp.tile([P2, 8, P2], F32, tag="pr")
                    nc.vector.tensor_reduce(
                        pr[0:P2, :, :],
                        tcb[0:P2, :, :].rearrange("p c (j m) -> p c j m", m=8),
                        mybir.AxisListType.X,
                        mybir.AluOpType.add,
                    )
                    nc.vector.tensor_scalar_mul(
                        pooled[0:P2, 8 * cc:8 * (cc + 1), :], pr[0:P2, :, :],
                        1.0 / 128.0,
                    )

            # ---------------- pass B: img (x) + attention -------------------
            with (
                tc.tile_pool(name="stageB", bufs=3) as stagepB,
                tc.tile_pool(name="mlpsB", bufs=2) as mlppB,
                tc.tile_pool(name="imgring", bufs=1) as imgp,
                tc.tile_pool(name="attev", bufs=4) as attevp,
                tc.tile_pool(name="psatt", bufs=2, space="PSUM") as psatt,
            ):
                for ring in range(NRING):
                    img = imgp.tile([P2, RING_B, 128], BF16, tag="img")
                    for sub in range(RING_B // SUB_B):
                        conv_mlp(
                            stagepB, mlppB, SUB_B, xp_d, wi_sb, bi_d,
                            ring * RING_B + sub * SUB_B, img, sub * SUB_B,
                            RING_B, sub,
                        )
                    lp0 = ring * RING_B
                    ncols = RING_B * 2
                    for c in range(C):
                        ap = psatt.tile([P2, 512], F32, tag="psa")
                        # rhs cols (slot, lp): l = 648*slot + lp0 + lp
                        rhs = img[0:P2, :, :].rearrange(
                            "p l (s c) -> p c s l", s=2
                        )[:, c:c + 1, :, :]
                        nc.tensor.matmul(
                            ap[0:P2, 0:ncols], pooled[:, c:c + 1, :], rhs,
                            start=True, stop=True,
                        )
                        ev = attevp.tile([P2, 2, RING_B], BF16, tag="attev")
                        src = ap[0:P2, 0:ncols].rearrange("p (s l) -> p s l", s=2)
                        nc.scalar.copy(ev[0:P2, :, :], src)
                        # out[e, c, 648*slot + lp0 : +RING_B]
                        dstap = out_d[0:P2, c:c + 1, :].rearrange(
                            "p o (s l) -> p o s l", s=2
                        )[:, :, :, lp0:lp0 + RING_B]
                        nc.sync.dma_start(out=dstap, in_=ev[0:P2, :, :])
    nc.compile()
    return nc


def _host_prep(x, y, w_img, b_img, w_fea, b_fea, w1, w2):
    f32 = np.float32
    bf16 = ml_dtypes.bfloat16
    weff = (w2.astype(np.float64) @ w1.astype(np.float64))  # (81, 81)
    wm = np.concatenate([weff.T, weff.sum(axis=1)[None, :]], axis=0)
    wm = wm.astype(f32).astype(bf16)

    def pairw(w):
        blk = np.zeros((128, 128), dtype=f32)
        blk[0:64, 0:64] = w.T
        blk[64:128, 64:128] = w.T
        return blk.astype(bf16)

    wi = pairw(w_img.astype(f32))
    wf = pairw(w_fea.astype(f32))
    # bias row in (sc, lp)-major order: value b[sc % 64] repeated NLP times
    bi = np.tile(np.concatenate([b_img, b_img]).astype(f32), SUB_B)[None, :]
    bf_ = np.tile(np.concatenate([b_fea, b_fea]).astype(f32), SUB_B)[None, :]
    bi = bi.astype(bf16)
    bf_ = bf_.astype(bf16)

    def unf_pairs(t):  # (1, 64, 72, 108, 108) -> per-core (128, 648, 81)
        u = np.ascontiguousarray(
            t.reshape(C, NCORES, P, LLOC, P).transpose(1, 0, 3, 2, 4)
        ).reshape(NCORES, C, LLOC, P2)
        out = []
        for k in range(NCORES):
            v = u[k].reshape(C, 2, LP, P2).transpose(1, 0, 2, 3)  # slot-halves
            out.append(np.ascontiguousarray(v.reshape(128, LP, P2)).astype(bf16))
        return out

    xps = unf_pairs(np.asarray(x, dtype=f32))
    yps = unf_pairs(np.asarray(y, dtype=f32))
    shared = {"wi": wi, "wf": wf, "wm": wm, "bi": bi, "bf": bf_}
    return [dict(shared, xp=xps[k], yp=yps[k]) for k in range(NCORES)]


def kernel(x, y, w_img, b_img, w_fea, b_fea, w1, w2):
    if "nc" not in _cache:
        _cache["nc"] = _build_nc()
    nc = _cache["nc"]
    in_maps = _host_prep(x, y, w_img, b_img, w_fea, b_fea, w1, w2)
    trace = bool(os.environ.get("KERNEL_TRACE"))
    res = run_bass_kernel_spmd(
        nc, in_maps, list(range(NCORES)), trace=trace
    )
    _cache["last_result"] = res
    out = np.empty((1, C, D, H, W), dtype=np.float32)
    ov = out.reshape(C, D, HW)
    for k in range(NCORES):
        # out_d is (81, 64, 1296) with l = 648*slot + lp (already global l)
        att = res.results[k]["out"].astype(np.float32).transpose(1, 2, 0)
        blk = att.reshape(C, LLOC, P, P).transpose(0, 2, 1, 3).reshape(C, P, HW)
        ov[:, P * k:P * (k + 1), :] = blk
    return out



# revision 4
# speedup vs baseline: 1.3136x; 1.3136x over previous
"""Trainium2 Bass kernel for nn_Cross_attention_3 (sparse_attention).

Sharding: spatial over the 10368 unfold patches.  The x-side (img) gives
core k patches [1296k, 1296k+1296) -- exactly one D-block of 9 rows, so
the fold/output stays local.  The y-side (fea -> pooled) instead gives
core k patches [1280k, 1280k+1408): aligned to the 128-patch pooling
bins (11 whole bins per core, neighbours overlap by one bin), so every
pooling bin is computed wholly on one core and the only collective is a
180KB-per-core AllGather of partial pooled bins, fully overlapped with
the x-side conv+MLP.

The two MLP linears collapse into one 81x81 matrix (no nonlinearity
between them); the conv bias rides as an 82nd contraction row whose ms
row holds b[c] (written once per pass, not per chunk).  The 1x1x1 conv
uses the patch data as the matmul's stationary operand so its output
lands directly in (patch-element, channel) layout -- the transpose the
MLP needs comes for free.  PSUM is evicted in 2-bank [81,1024] strides,
conv evicts split across DVE/ACT, MLP+LeakyReLU on ACT, the pooling
tree on DVE at bf16 2x rate, attention evicted two channels per PSUM
tile with output DMAs on the otherwise idle gpsimd queue.
"""

import os
import sys

import numpy as np

try:
    import ml_dtypes
except ImportError:
    ml_dtypes = None

try:
    import concourse.bacc as _  # noqa: F401
except ImportError:  # container default path
    sys.path.insert(0, "/opt/trn_rl_repo")

import concourse.bacc as bacc
import concourse.mybir as mybir
from concourse.bass_utils import run_bass_kernel_spmd
from concourse.tile import TileContext

P = 9
P2 = 81
C = 64
D = 72
H = W = 108
HW = H * W
NCORES = 8

# x-side: exact shard, 1296 patches = 648 slot pairs per core
LX = 1296
LPX = LX // 2          # 648
RING = 216             # attention ring, in pairs
NRING = LPX // RING    # 3
NLPB = 24              # pass-B chunk, in pairs
NCHB = RING // NLPB    # 9 chunks per ring

# y-side: bin-aligned shard with overlap, 1408 patches = 704 pairs
LY = 1408
LPY = LY // 2          # 704
NLPA = 32              # pass-A chunk, in pairs
NCHA = LPY // NLPA     # 22
NBIN = 11              # local pooling bins per core (128 patches each)
NU = 22                # 32-patch units per slot (704/32)

F32 = mybir.dt.float32
BF16 = mybir.dt.bfloat16

_cache = {}


def _build_nc():
    nc = bacc.Bacc(None, target_bir_lowering=False, debug=False)
    xp_d = nc.declare_dram_parameter("xp", [128, LPX, P2], BF16, isOutput=False)
    yp_d = nc.declare_dram_parameter("yp", [128, LPY, P2], BF16, isOutput=False)
    wi_d = nc.declare_dram_parameter("wi", [128, 128], BF16, isOutput=False)
    wf_d = nc.declare_dram_parameter("wf", [128, 128], BF16, isOutput=False)
    wm_d = nc.declare_dram_parameter("wm", [82, P2], BF16, isOutput=False)
    bi_d = nc.declare_dram_parameter("bi", [1, NLPA * 128], BF16, isOutput=False)
    bf_d = nc.declare_dram_parameter("bf", [1, NLPA * 128], BF16, isOutput=False)
    out_d = nc.declare_dram_parameter("out", [P2, C, LX], BF16, isOutput=True)

    # per-core partial pooled bins, gathered across cores
    gath_d = nc.dram_tensor("gath", [NCORES, P2, NBIN, C], F32,
                            addr_space="Shared")

    with nc.allow_low_precision("bf16 compute pipeline"), TileContext(nc) as tc:
        with (
            tc.tile_pool(name="const", bufs=1) as constp,
            tc.tile_pool(name="stage", bufs=3) as stagep,
            tc.tile_pool(name="feap", bufs=2) as feap,
            tc.tile_pool(name="treep", bufs=1) as treep,
            tc.tile_pool(name="imgp", bufs=1) as imgp,
            tc.tile_pool(name="evp", bufs=4) as evp,
            tc.tile_pool(name="ps", bufs=2, space="PSUM") as psp,
            tc.tile_pool(name="dram", bufs=1, space="DRAM") as dramp,
        ):
            wi_sb = constp.tile([128, 128], BF16, tag="wi")
            wf_sb = constp.tile([128, 128], BF16, tag="wf")
            wm_sb = constp.tile([82, P2], BF16, tag="wm")
            ms0 = constp.tile([82, NLPA, 128], BF16, tag="ms0")
            ms1 = constp.tile([82, NLPA, 128], BF16, tag="ms1")
            h32 = constp.tile([P2, 2, NU, C], F32, tag="h32")
            b1t = constp.tile([P2, NU, C], F32, tag="b1t")
            part = constp.tile([P2, NBIN, C], F32, tag="part")
            pooled = constp.tile([P2, C, P2], BF16, tag="pooled")
            stg = constp.tile([P2, NCORES, NBIN, C], F32, tag="stg")
            part_dram = dramp.tile([P2, NBIN, C], F32)

            nc.sync.dma_start(out=wi_sb[:, :], in_=wi_d[:, :])
            nc.sync.dma_start(out=wf_sb[:, :], in_=wf_d[:, :])
            nc.sync.dma_start(out=wm_sb[:, :], in_=wm_d[:, :])
            # conv-bias contraction rows, written once per pass (pass B
            # rewrites them with bi after the last pass-A MLP consumes bf)
            nc.sync.dma_start(
                out=ms0[81:82, :, :].rearrange("p a b -> p (a b)"), in_=bf_d[:, :]
            )
            nc.sync.dma_start(
                out=ms1[81:82, :, :].rearrange("p a b -> p (a b)"), in_=bf_d[:, :]
            )

            # ---------------- pass A: fea (y) + local pooled bins -----------
            for ch in range(NCHA):
                st = stagep.tile([128, NLPA, P2], BF16, tag="st")
                nc.sync.dma_start(
                    out=st[:, :, :], in_=yp_d[:, NLPA * ch:NLPA * (ch + 1), :]
                )
                ms = ms0 if ch % 2 == 0 else ms1
                # conv: 8 pairs per 2-bank PSUM tile, single strided evict
                for t in range(NLPA // 8):
                    cv = psp.tile([P2, 1024], F32, tag="cv")
                    for j in range(8):
                        nc.tensor.matmul(
                            cv[0:P2, 128 * j:128 * (j + 1)],
                            st[:, 8 * t + j, :], wf_sb[:, :],
                            start=True, stop=True,
                        )
                    d = ms[0:P2, 8 * t:8 * t + 8, :].rearrange("p a b -> p (a b)")
                    if t < 2:
                        nc.vector.tensor_copy(d, cv[0:P2, :])
                    else:
                        nc.scalar.copy(d, cv[0:P2, :])
                # MLP (+bias row) and LeakyReLU on ACT
                fea = feap.tile([P2, NLPA, 128], BF16, tag="fea")
                flat = ms[:, :, :].rearrange("p a b -> p (a b)")
                for q in range(NLPA // 8):
                    mp = psp.tile([P2, 1024], F32, tag="ml")
                    for g in range(2):
                        nc.tensor.matmul(
                            mp[0:P2, 512 * g:512 * (g + 1)],
                            wm_sb[:, :],
                            flat[:, 1024 * q + 512 * g:1024 * q + 512 * (g + 1)],
                            start=True, stop=True,
                        )
                    dq = fea[0:P2, 8 * q:8 * q + 8, :].rearrange("p a b -> p (a b)")
                    nc.scalar.activation(
                        dq, mp[0:P2, :],
                        mybir.ActivationFunctionType.Prelu, alpha=0.2,
                    )
                # pairwise tree over lp: 32 -> 16 -> 8 -> 4 -> 2 -> 1
                # (bf16 2x DVE), final 32-patch sums land in h32 as f32
                t1 = treep.tile([P2, 16, 128], BF16, tag="t1")
                f2 = fea[0:P2, :, :].rearrange("p (a two) b -> p a two b", two=2)
                nc.vector.tensor_tensor(
                    t1[0:P2, :, :], f2[:, :, 0, :], f2[:, :, 1, :],
                    mybir.AluOpType.add,
                )
                t2 = treep.tile([P2, 8, 128], BF16, tag="t2")
                t1s = t1[0:P2, :, :].rearrange("p (a two) b -> p a two b", two=2)
                nc.vector.tensor_tensor(
                    t2[0:P2, :, :], t1s[:, :, 0, :], t1s[:, :, 1, :],
                    mybir.AluOpType.add,
                )
                t3 = treep.tile([P2, 4, 128], BF16, tag="t3")
                t2s = t2[0:P2, :, :].rearrange("p (a two) b -> p a two b", two=2)
                nc.vector.tensor_tensor(
                    t3[0:P2, :, :], t2s[:, :, 0, :], t2s[:, :, 1, :],
                    mybir.AluOpType.add,
                )
                t4 = treep.tile([P2, 2, 128], BF16, tag="t4")
                t3s = t3[0:P2, :, :].rearrange("p (a two) b -> p a two b", two=2)
                nc.vector.tensor_tensor(
                    t4[0:P2, :, :], t3s[:, :, 0, :], t3s[:, :, 1, :],
                    mybir.AluOpType.add,
                )
                nc.vector.tensor_tensor(
                    h32[0:P2, :, ch, :],
                    t4[0:P2, 0, :].rearrange("p (s c) -> p s c", s=2),
                    t4[0:P2, 1, :].rearrange("p (s c) -> p s c", s=2),
                    mybir.AluOpType.add,
                )

            # bin tree: 44 unit sums (t = 22*slot + u, consecutive global
            # 32-patch units) -> 22 -> 11 bins of 128
            hf = h32[0:P2, :, :, :].rearrange("p s u c -> p (s u) c")
            hp = hf.rearrange("p (a two) c -> p a two c", two=2)
            nc.vector.tensor_tensor(
                b1t[0:P2, :, :], hp[:, :, 0, :], hp[:, :, 1, :],
                mybir.AluOpType.add,
            )
            b1p = b1t[0:P2, :, :].rearrange("p (a two) c -> p a two c", two=2)
            nc.vector.tensor_tensor(
                part[0:P2, :, :], b1p[:, :, 0, :], b1p[:, :, 1, :],
                mybir.AluOpType.add,
            )
            nc.gpsimd.dma_start(out=part_dram[:, :, :], in_=part[:, :, :])
            nc.gpsimd.collective_compute(
                "AllGather",
                mybir.AluOpType.bypass,
                replica_groups=[list(range(NCORES))],
                ins=[part_dram[:, :, :]],
                outs=[gath_d[:, :, :, :]],
            )

            # rewrite bias rows for pass B
            nc.sync.dma_start(
                out=ms0[81:82, :, :].rearrange("p a b -> p (a b)"), in_=bi_d[:, :]
            )
            nc.sync.dma_start(
                out=ms1[81:82, :, :].rearrange("p a b -> p (a b)"), in_=bi_d[:, :]
            )

            # ---------------- pass B: img (x) + attention -------------------
            msc = 0
            for r in range(NRING):
                img = imgp.tile([P2, RING, 128], BF16, tag="img")
                for chb in range(NCHB):
                    lp0 = RING * r + NLPB * chb
                    st = stagep.tile([128, NLPA, P2], BF16, tag="st")
                    nc.sync.dma_start(
                        out=st[:, 0:NLPB, :], in_=xp_d[:, lp0:lp0 + NLPB, :]
                    )
                    ms = ms0 if msc % 2 == 0 else ms1
                    msc += 1
                    for t in range(NLPB // 8):
                        cv = psp.tile([P2, 1024], F32, tag="cv")
                        for j in range(8):
                            nc.tensor.matmul(
                                cv[0:P2, 128 * j:128 * (j + 1)],
                                st[:, 8 * t + j, :], wi_sb[:, :],
                                start=True, stop=True,
                            )
                        d = ms[0:P2, 8 * t:8 * t + 8, :].rearrange(
                            "p a b -> p (a b)")
                        if t < 2:
                            nc.vector.tensor_copy(d, cv[0:P2, :])
                        else:
                            nc.scalar.copy(d, cv[0:P2, :])
                    flat = ms[:, :, :].rearrange("p a b -> p (a b)")
                    for t in range(NLPB // 8):
                        mp = psp.tile([P2, 1024], F32, tag="ml")
                        for g in range(2):
                            nc.tensor.matmul(
                                mp[0:P2, 512 * g:512 * (g + 1)],
                                wm_sb[:, :],
                                flat[:, 1024 * t + 512 * g:
                                     1024 * t + 512 * (g + 1)],
                                start=True, stop=True,
                            )
                        dq = img[0:P2, NLPB * chb + 8 * t:NLPB * chb + 8 * t + 8,
                                 :].rearrange("p a b -> p (a b)")
                        nc.scalar.activation(
                            dq, mp[0:P2, :],
                            mybir.ActivationFunctionType.Prelu, alpha=0.2,
                        )
                if r == 0:
                    # assemble pooled from the gathered per-core bins
                    # (collective long since done; overlapped with ring 0)
                    nc.gpsimd.dma_start(
                        out=stg[:, :, :, :],
                        in_=gath_d[:, :, :, :].rearrange("k p b c -> p k b c"),
                    )
                    nc.vector.tensor_scalar_mul(
                        pooled[0:P2, :, 0:80].rearrange(
                            "p c (k b) -> p k b c", k=8),
                        stg[0:P2, :, 0:10, :], 1.0 / 128.0,
                    )
                    nc.vector.tensor_scalar_mul(
                        pooled[0:P2, :, 80:81].rearrange("p c j -> p j c"),
                        stg[0:P2, 7:8, 10:11, :].rearrange(
                            "p k b c -> p (k b) c"),
                        1.0 / 128.0,
                    )
                # attention for this ring: two channels per PSUM tile
                lp0 = RING * r
                rhsv = img[0:P2, :, :].rearrange("p l (s c) -> p c s l", s=2)
                for i in range(C // 2):
                    ap = psp.tile([P2, 1024], F32, tag="ml")
                    for g in range(2):
                        c = 2 * i + g
                        nc.tensor.matmul(
                            ap[0:P2, 512 * g:512 * g + 2 * RING],
                            pooled[:, c:c + 1, :], rhsv[:, c:c + 1, :, :],
                            start=True, stop=True,
                        )
                    ev = evp.tile([P2, 2, 2, RING], BF16, tag="ev")
                    src = ap[0:P2, :].rearrange(
                        "p (g x) -> p g x", g=2)[:, :, 0:2 * RING].rearrange(
                        "p g (s l) -> p g s l", s=2)
                    if i % 2 == 0:
                        nc.vector.tensor_copy(ev[0:P2, :, :, :], src)
                    else:
                        nc.scalar.copy(ev[0:P2, :, :, :], src)
                    dst = out_d[0:P2, 2 * i:2 * i + 2, :].rearrange(
                        "p c (s l) -> p c s l", s=2)[:, :, :, lp0:lp0 + RING]
                    if i % 2 == 0:
                        nc.gpsimd.dma_start(out=dst, in_=ev[0:P2, :, :, :])
                    else:
                        nc.sync.dma_start(out=dst, in_=ev[0:P2, :, :, :])
    nc.compile()
    return nc


def _host_prep(x, y, w_img, b_img, w_fea, b_fea, w1, w2):
    f32 = np.float32
    bf16 = ml_dtypes.bfloat16
    weff = (w2.astype(np.float64) @ w1.astype(np.float64))  # (81, 81)
    wm = np.concatenate([weff.T, weff.sum(axis=1)[None, :]], axis=0)
    wm = wm.astype(f32).astype(bf16)

    def pairw(w):
        blk = np.zeros((128, 128), dtype=f32)
        blk[0:64, 0:64] = w.T
        blk[64:128, 64:128] = w.T
        return blk.astype(bf16)

    wi = pairw(w_img.astype(f32))
    wf = pairw(w_fea.astype(f32))
    bi = np.tile(np.concatenate([b_img, b_img]).astype(f32), NLPA)[None, :]
    bf_ = np.tile(np.concatenate([b_fea, b_fea]).astype(f32), NLPA)[None, :]
    bi = bi.astype(bf16)
    bf_ = bf_.astype(bf16)

    def unfold(t):  # (1, 64, 72, 108, 108) -> (C, 10368, 81) patch matrix
        u = np.ascontiguousarray(
            t.reshape(C, D // P, P, HW // P, P).transpose(0, 1, 3, 2, 4)
        ).reshape(C, (D // P) * (HW // P), P2)
        return u

    def pack(u, l0, lhalf):  # global patches [l0, l0+2*lhalf) -> [128, lhalf, 81]
        v = u[:, l0:l0 + 2 * lhalf, :].reshape(C, 2, lhalf, P2)
        v = v.transpose(1, 0, 2, 3).reshape(128, lhalf, P2)
        return np.ascontiguousarray(v).astype(ml_dtypes.bfloat16)

    ux = unfold(np.asarray(x, dtype=f32))
    uy = unfold(np.asarray(y, dtype=f32))
    shared = {"wi": wi, "wf": wf, "wm": wm, "bi": bi, "bf": bf_}
    maps = []
    for k in range(NCORES):
        maps.append(dict(
            shared,
            xp=pack(ux, LX * k, LPX),
            yp=pack(uy, (LY - 128) * k, LPY),
        ))
    return maps


def kernel(x, y, w_img, b_img, w_fea, b_fea, w1, w2):
    if "nc" not in _cache:
        _cache["nc"] = _build_nc()
    nc = _cache["nc"]
    in_maps = _host_prep(x, y, w_img, b_img, w_fea, b_fea, w1, w2)
    trace = bool(os.environ.get("KERNEL_TRACE"))
    res = run_bass_kernel_spmd(
        nc, in_maps, list(range(NCORES)), trace=trace
    )
    _cache["last_result"] = res
    out = np.empty((1, C, D, H, W), dtype=np.float32)
    ov = out.reshape(C, D, HW)
    for k in range(NCORES):
        # out_d is (81, 64, 1296) with l = 648*slot + lp (already global l)
        att = res.results[k]["out"].astype(np.float32).transpose(1, 2, 0)
        blk = att.reshape(C, LX, P, P).transpose(0, 2, 1, 3).reshape(C, P, HW)
        ov[:, P * k:P * (k + 1), :] = blk
    return out


# revision 9
# speedup vs baseline: 1.3215x; 1.0061x over previous
"""Trainium2 Bass kernel for nn_Cross_attention_3 (sparse_attention).

Sharding: spatial over the 10368 unfold patches.  The x-side (img) gives
core k patches [1296k, 1296k+1296) -- one D-block of 9 rows, so the
fold/output stays local.  The y-side (fea -> pooled) gives core k
patches [1280k, 1280k+1408): aligned to the 128-patch pooling bins (11
whole bins per core, neighbours overlap by one bin), so every pooling
bin is computed wholly on one core and the only collective is a 90KB
AllGather of partial pooled bins, fully overlapped with the x-side.

The two MLP linears collapse into one 81x81 matrix; the conv bias rides
as an 82nd contraction row written once per pass.  The 1x1x1 conv uses
the patch data as the matmul's stationary operand so its output lands
directly in (patch-element, channel) layout.  PSUM is evicted in 2-bank
[81,1024] strides; evictions (conv copy, MLP LeakyReLU via
max(0.2x,x), attention copy) are load-balanced across DVE, ACT and
GpSimd.  Attention matmuls for ring r are interleaved into ring r+1's
conv/MLP chunk stream to keep the PE densely fed (p-state ramp) and to
spread eviction load.
"""

import os
import sys

import numpy as np

try:
    import ml_dtypes
except ImportError:
    ml_dtypes = None

try:
    import concourse.bacc as _  # noqa: F401
except ImportError:  # container default path
    sys.path.insert(0, "/opt/trn_rl_repo")

import concourse.bacc as bacc
import concourse.mybir as mybir
from concourse.bass_utils import run_bass_kernel_spmd
from concourse.tile import TileContext

P = 9
P2 = 81
C = 64
D = 72
H = W = 108
HW = H * W
NCORES = 8

# x-side: exact shard, 1296 patches = 648 slot pairs per core
LX = 1296
LPX = LX // 2          # 648
RING = 216             # attention ring, in pairs
NRING = LPX // RING    # 3
NLPB = 24              # pass-B chunk, in pairs
NCHB = RING // NLPB    # 9 chunks per ring
NATT = C // 2          # att tiles (2 channels each) per ring

# y-side: bin-aligned shard with overlap, 1408 patches = 704 pairs
LY = 1408
LPY = LY // 2          # 704
NLPA = 32              # pass-A chunk, in pairs
NCHA = LPY // NLPA     # 22
NBIN = 11              # local pooling bins per core (128 patches each)
NU = 22                # 32-patch units per slot (704/32)

F32 = mybir.dt.float32
BF16 = mybir.dt.bfloat16

_cache = {}


def _build_nc():
    nc = bacc.Bacc(None, target_bir_lowering=False, debug=False)
    xp_d = nc.declare_dram_parameter("xp", [128, LPX, P2], BF16, isOutput=False)
    yp_d = nc.declare_dram_parameter("yp", [128, LPY, P2], BF16, isOutput=False)
    wi_d = nc.declare_dram_parameter("wi", [128, 128], BF16, isOutput=False)
    wf_d = nc.declare_dram_parameter("wf", [128, 128], BF16, isOutput=False)
    wm_d = nc.declare_dram_parameter("wm", [82, P2], BF16, isOutput=False)
    bi_d = nc.declare_dram_parameter("bi", [1, NLPA * 128], BF16, isOutput=False)
    bf_d = nc.declare_dram_parameter("bf", [1, NLPA * 128], BF16, isOutput=False)
    out_d = nc.declare_dram_parameter("out", [P2, C, LX], BF16, isOutput=True)

    gath_d = nc.dram_tensor("gath", [NCORES, P2, NBIN, C], BF16,
                            addr_space="Shared")

    MUL = mybir.AluOpType.mult
    MAX = mybir.AluOpType.max
    ADD = mybir.AluOpType.add

    def evict(eng, dst, src, lrelu=False):
        if lrelu:
            assert eng == "a"
            nc.scalar.activation(dst, src, mybir.ActivationFunctionType.Prelu,
                                 alpha=0.2)
        elif eng == "a":
            nc.scalar.copy(dst, src)
        else:
            e = {"v": nc.vector, "g": nc.gpsimd}[eng]
            e.tensor_copy(dst, src)

    with nc.allow_low_precision("bf16 compute pipeline"), TileContext(nc) as tc:
        with (
            tc.tile_pool(name="const", bufs=1) as constp,
            tc.tile_pool(name="stage", bufs=2) as stagep,
            tc.tile_pool(name="feap", bufs=2) as feap,
            tc.tile_pool(name="treep", bufs=1) as treep,
            tc.tile_pool(name="imgp", bufs=2) as imgp,
            tc.tile_pool(name="evp", bufs=3) as evp,
            tc.tile_pool(name="ps", bufs=2, space="PSUM") as psp,
            tc.tile_pool(name="dram", bufs=1, space="DRAM") as dramp,
        ):
            wi_sb = constp.tile([128, 128], BF16, tag="wi")
            wf_sb = constp.tile([128, 128], BF16, tag="wf")
            wm_sb = constp.tile([82, P2], BF16, tag="wm")
            ms0 = constp.tile([82, NLPA, 128], BF16, tag="ms0")
            ms1 = constp.tile([82, NLPA, 128], BF16, tag="ms1")
            h32 = constp.tile([P2, 2, NU, C], BF16, tag="h32")
            b1t = constp.tile([P2, NU, C], BF16, tag="b1t")
            part = constp.tile([P2, NBIN, C], BF16, tag="part")
            pooled = constp.tile([P2, C, P2], BF16, tag="pooled")
            stg = constp.tile([P2, NCORES, NBIN, C], BF16, tag="stg")
            part_dram = dramp.tile([P2, NBIN, C], BF16)

            nc.sync.dma_start(out=wi_sb[:, :], in_=wi_d[:, :])
            nc.sync.dma_start(out=wf_sb[:, :], in_=wf_d[:, :])
            nc.sync.dma_start(out=wm_sb[:, :], in_=wm_d[:, :])
            # conv-bias contraction rows, written once per pass
            nc.sync.dma_start(
                out=ms0[81:82, :, :].rearrange("p a b -> p (a b)"), in_=bf_d[:, :]
            )
            nc.sync.dma_start(
                out=ms1[81:82, :, :].rearrange("p a b -> p (a b)"), in_=bf_d[:, :]
            )

            def conv_chunk(st_src, lp0, nlp, w_sb, ms, engs):
                """stage DMA + conv matmuls + evicts into ms rows 0..80."""
                st = stagep.tile([128, NLPA, P2], BF16, tag="st")
                nc.sync.dma_start(
                    out=st[:, 0:nlp, :], in_=st_src[:, lp0:lp0 + nlp, :]
                )
                for t in range(nlp // 8):
                    cv = psp.tile([P2, 1024], F32, tag="cv")
                    for j in range(8):
                        nc.tensor.matmul(
                            cv[0:P2, 128 * j:128 * (j + 1)],
                            st[:, 8 * t + j, :], w_sb[:, :],
                            start=True, stop=True,
                        )
                    d = ms[0:P2, 8 * t:8 * t + 8, :].rearrange("p a b -> p (a b)")
                    evict(engs[t], d, cv[0:P2, :])

            def mlp_tiles(ms, nlp, dst, dst_lp0, engs):
                """MLP matmuls + ACT Prelu evicts into dst at pair offset."""
                flat = ms[:, :, :].rearrange("p a b -> p (a b)")
                for t in range(nlp // 8):
                    mp = psp.tile([P2, 1024], F32, tag="ml")
                    for g in range(2):
                        nc.tensor.matmul(
                            mp[0:P2, 512 * g:512 * (g + 1)],
                            wm_sb[:, :],
                            flat[:, 1024 * t + 512 * g:
                                 1024 * t + 512 * (g + 1)],
                            start=True, stop=True,
                        )
                    dq = dst[0:P2, dst_lp0 + 8 * t:dst_lp0 + 8 * t + 8,
                             :].rearrange("p a b -> p (a b)")
                    evict(engs[t], dq, mp[0:P2, :], lrelu=True)

            # ---------------- pass A: fea (y) + local pooled bins -----------
            for ch in range(NCHA):
                ms = ms0 if ch % 2 == 0 else ms1
                # conv evicts: DVE, ACT, ACT, GP; mlp: ACT, GP, GP + 1 ACT
                conv_chunk(yp_d, NLPA * ch, NLPA, wf_sb, ms,
                           ["v", "a", "v", "v"] if ch % 2 == 0 else
                           ["v", "v", "v", "v"])
                fea = feap.tile([P2, NLPA, 128], BF16, tag="fea")
                mlp_tiles(ms, NLPA, fea, 0, ["a", "a", "a", "a"])
                # pairwise tree over lp: 32 -> 16 -> 8 -> 4 -> 2 -> 1
                # (bf16 2x DVE); 32-patch sums land in h32
                t1 = treep.tile([P2, 16, 128], BF16, tag="t1")
                f2 = fea[0:P2, :, :].rearrange("p (a two) b -> p a two b", two=2)
                nc.vector.tensor_tensor(
                    t1[0:P2, :, :], f2[:, :, 0, :], f2[:, :, 1, :], ADD)
                t2 = treep.tile([P2, 8, 128], BF16, tag="t2")
                t1s = t1[0:P2, :, :].rearrange("p (a two) b -> p a two b", two=2)
                nc.gpsimd.tensor_tensor(
                    t2[0:P2, :, :], t1s[:, :, 0, :], t1s[:, :, 1, :], ADD)
                t3 = treep.tile([P2, 4, 128], BF16, tag="t3")
                t2s = t2[0:P2, :, :].rearrange("p (a two) b -> p a two b", two=2)
                nc.gpsimd.tensor_tensor(
                    t3[0:P2, :, :], t2s[:, :, 0, :], t2s[:, :, 1, :], ADD)
                t4 = treep.tile([P2, 2, 128], BF16, tag="t4")
                t3s = t3[0:P2, :, :].rearrange("p (a two) b -> p a two b", two=2)
                nc.gpsimd.tensor_tensor(
                    t4[0:P2, :, :], t3s[:, :, 0, :], t3s[:, :, 1, :], ADD)
                nc.gpsimd.tensor_tensor(
                    h32[0:P2, :, ch, :],
                    t4[0:P2, 0, :].rearrange("p (s c) -> p s c", s=2),
                    t4[0:P2, 1, :].rearrange("p (s c) -> p s c", s=2),
                    ADD)

            # bin tree: 44 unit sums (t = 22*slot + u are consecutive global
            # 32-patch units) -> 22 -> 11 bins of 128 patches
            hf = h32[0:P2, :, :, :].rearrange("p s u c -> p (s u) c")
            hp = hf.rearrange("p (a two) c -> p a two c", two=2)
            nc.gpsimd.tensor_tensor(
                b1t[0:P2, :, :], hp[:, :, 0, :], hp[:, :, 1, :], ADD)
            b1p = b1t[0:P2, :, :].rearrange("p (a two) c -> p a two c", two=2)
            nc.gpsimd.tensor_tensor(
                part[0:P2, :, :], b1p[:, :, 0, :], b1p[:, :, 1, :], ADD)
            nc.gpsimd.dma_start(out=part_dram[:, :, :], in_=part[:, :, :])
            nc.gpsimd.collective_compute(
                "AllGather",
                mybir.AluOpType.bypass,
                replica_groups=[list(range(NCORES))],
                ins=[part_dram[:, :, :]],
                outs=[gath_d[:, :, :, :]],
            )

            # rewrite bias rows for pass B
            nc.sync.dma_start(
                out=ms0[81:82, :, :].rearrange("p a b -> p (a b)"), in_=bi_d[:, :]
            )
            nc.sync.dma_start(
                out=ms1[81:82, :, :].rearrange("p a b -> p (a b)"), in_=bi_d[:, :]
            )

            # ---------------- pass B: img (x) + attention -------------------
            def att_tiles(img, r_att, i0, n, ev_engs, dma_engs):
                lp0 = RING * r_att
                rhsv = img[0:P2, :, :].rearrange("p l (s c) -> p c s l", s=2)
                for i in range(i0, i0 + n):
                    ap = psp.tile([P2, 1024], F32, tag="ml")
                    for g in range(2):
                        c = 2 * i + g
                        nc.tensor.matmul(
                            ap[0:P2, 512 * g:512 * g + 2 * RING],
                            pooled[:, c:c + 1, :], rhsv[:, c:c + 1, :, :],
                            start=True, stop=True,
                        )
                    ev = evp.tile([P2, 2, 2, RING], BF16, tag="ev")
                    src = ap[0:P2, :].rearrange(
                        "p (g x) -> p g x", g=2)[:, :, 0:2 * RING].rearrange(
                        "p g (s l) -> p g s l", s=2)
                    evict(ev_engs[i - i0], ev[0:P2, :, :, :], src)
                    dst = out_d[0:P2, 2 * i:2 * i + 2, :].rearrange(
                        "p c (s l) -> p c s l", s=2)[:, :, :, lp0:lp0 + RING]
                    if dma_engs[i - i0] == "g":
                        nc.gpsimd.dma_start(out=dst, in_=ev[0:P2, :, :, :])
                    else:
                        nc.sync.dma_start(out=dst, in_=ev[0:P2, :, :, :])

            imgs = []
            msc = 0
            for r in range(NRING):
                img = imgp.tile([P2, RING, 128], BF16, tag="img")
                imgs.append(img)
                for chb in range(NCHB):
                    ms = ms0 if msc % 2 == 0 else ms1
                    msc += 1
                    if r == 0:
                        cengs = ["v", "v", "v"]
                    else:
                        cengs = ["v", "v", "a"]
                    mengs = ["a", "a", "a"]
                    conv_chunk(xp_d, RING * r + NLPB * chb, NLPB, wi_sb, ms,
                               cengs)
                    mlp_tiles(ms, NLPB, img, NLPB * chb, mengs)
                    if r >= 1 and chb < 8:
                        att_tiles(imgs[r - 1], r - 1, 4 * chb, 4,
                                  ["v", "v", "v", "a"],
                                  ["s", "s", "s", "s"])
                if r == 0:
                    # pooled assembly (collective long since done)
                    nc.gpsimd.dma_start(
                        out=stg[:, :, :, :],
                        in_=gath_d[:, :, :, :].rearrange("k p b c -> p k b c"),
                    )
                    nc.vector.tensor_scalar_mul(
                        pooled[0:P2, :, 0:80].rearrange(
                            "p c (k b) -> p k b c", k=8),
                        stg[0:P2, :, 0:10, :], 1.0 / 128.0,
                    )
                    nc.vector.tensor_scalar_mul(
                        pooled[0:P2, :, 80:81].rearrange("p c j -> p j c"),
                        stg[0:P2, 7:8, 10:11, :].rearrange(
                            "p k b c -> p (k b) c"),
                        1.0 / 128.0,
                    )
            # trailing attention for the last ring
            ev_engs = [["v", "a"][i % 2] for i in range(NATT)]
            dma_engs = ["g" if i % 4 == 1 else "s" for i in range(NATT)]
            att_tiles(imgs[2], 2, 0, NATT, ev_engs, dma_engs)
    nc.compile()
    return nc


def _host_prep(x, y, w_img, b_img, w_fea, b_fea, w1, w2):
    f32 = np.float32
    bf16 = ml_dtypes.bfloat16
    weff = (w2.astype(np.float64) @ w1.astype(np.float64))  # (81, 81)
    wm = np.concatenate([weff.T, weff.sum(axis=1)[None, :]], axis=0)
    wm = wm.astype(f32).astype(bf16)

    def pairw(w):
        blk = np.zeros((128, 128), dtype=f32)
        blk[0:64, 0:64] = w.T
        blk[64:128, 64:128] = w.T
        return blk.astype(bf16)

    wi = pairw(w_img.astype(f32))
    wf = pairw(w_fea.astype(f32))
    bi = np.tile(np.concatenate([b_img, b_img]).astype(f32), NLPA)[None, :]
    bf_ = np.tile(np.concatenate([b_fea, b_fea]).astype(f32), NLPA)[None, :]
    bi = bi.astype(bf16)
    bf_ = bf_.astype(bf16)

    def unfold(t):  # (1, 64, 72, 108, 108) -> (C, 10368, 81) patch matrix
        u = np.ascontiguousarray(
            t.reshape(C, D // P, P, HW // P, P).transpose(0, 1, 3, 2, 4)
        ).reshape(C, (D // P) * (HW // P), P2)
        return u

    def pack(u, l0, lhalf):  # global patches [l0, l0+2*lhalf) -> [128, lhalf, 81]
        v = u[:, l0:l0 + 2 * lhalf, :].reshape(C, 2, lhalf, P2)
        v = v.transpose(1, 0, 2, 3).reshape(128, lhalf, P2)
        return np.ascontiguousarray(v).astype(ml_dtypes.bfloat16)

    ux = unfold(np.asarray(x, dtype=f32))
    uy = unfold(np.asarray(y, dtype=f32))
    shared = {"wi": wi, "wf": wf, "wm": wm, "bi": bi, "bf": bf_}
    maps = []
    for k in range(NCORES):
        maps.append(dict(
            shared,
            xp=pack(ux, LX * k, LPX),
            yp=pack(uy, (LY - 128) * k, LPY),
        ))
    return maps


def kernel(x, y, w_img, b_img, w_fea, b_fea, w1, w2):
    if "nc" not in _cache:
        _cache["nc"] = _build_nc()
    nc = _cache["nc"]
    in_maps = _host_prep(x, y, w_img, b_img, w_fea, b_fea, w1, w2)
    trace = bool(os.environ.get("KERNEL_TRACE"))
    res = run_bass_kernel_spmd(
        nc, in_maps, list(range(NCORES)), trace=trace
    )
    _cache["last_result"] = res
    out = np.empty((1, C, D, H, W), dtype=np.float32)
    ov = out.reshape(C, D, HW)
    for k in range(NCORES):
        # out_d is (81, 64, 1296) with l = 648*slot + lp (already global l)
        att = res.results[k]["out"].astype(np.float32).transpose(1, 2, 0)
        blk = att.reshape(C, LX, P, P).transpose(0, 2, 1, 3).reshape(C, P, HW)
        ov[:, P * k:P * (k + 1), :] = blk
    return out


# revision 10
# speedup vs baseline: 1.3307x; 1.0070x over previous
"""Trainium2 Bass kernel for nn_Cross_attention_3 (sparse_attention).

Sharding: spatial over the 10368 unfold patches.  The x-side (img) gives
core k patches [1296k, 1296k+1296) -- one D-block of 9 rows, so the
fold/output stays local.  The y-side (fea -> pooled) gives core k
patches [1280k, 1280k+1408): aligned to the 128-patch pooling bins (11
whole bins per core, neighbours overlap by one bin), so every pooling
bin is computed wholly on one core and the only collective is a 90KB
AllGather of partial pooled bins, fully overlapped with the x-side.

The two MLP linears collapse into one 81x81 matrix; the conv bias rides
as an 82nd contraction row written once per pass.  The 1x1x1 conv uses
the patch data as the matmul's stationary operand so its output lands
directly in (patch-element, channel) layout.  PSUM is evicted in 2-bank
[81,1024] strides; evictions (conv copy, MLP LeakyReLU via
max(0.2x,x), attention copy) are load-balanced across DVE, ACT and
GpSimd.  Attention matmuls for ring r are interleaved into ring r+1's
conv/MLP chunk stream to keep the PE densely fed (p-state ramp) and to
spread eviction load.
"""

import os
import sys

import numpy as np

try:
    import ml_dtypes
except ImportError:
    ml_dtypes = None

try:
    import concourse.bacc as _  # noqa: F401
except ImportError:  # container default path
    sys.path.insert(0, "/opt/trn_rl_repo")

import concourse.bacc as bacc
import concourse.mybir as mybir
from concourse.bass_utils import run_bass_kernel_spmd
from concourse.tile import TileContext

P = 9
P2 = 81
C = 64
D = 72
H = W = 108
HW = H * W
NCORES = 8

# x-side: exact shard, 1296 patches = 648 slot pairs per core
LX = 1296
LPX = LX // 2          # 648
RING = 216             # attention ring, in pairs
NRING = LPX // RING    # 3
NLPB = 24              # pass-B chunk, in pairs
NCHB = RING // NLPB    # 9 chunks per ring
NATT = C // 2          # att tiles (2 channels each) per ring

# y-side: bin-aligned shard with overlap, 1408 patches = 704 pairs
LY = 1408
LPY = LY // 2          # 704
NLPA = 32              # pass-A chunk, in pairs
NCHA = LPY // NLPA     # 22
NBIN = 11              # local pooling bins per core (128 patches each)
NU = 22                # 32-patch units per slot (704/32)

F32 = mybir.dt.float32
BF16 = mybir.dt.bfloat16

_cache = {}


def _build_nc():
    nc = bacc.Bacc(None, target_bir_lowering=False, debug=False)
    xp_d = nc.declare_dram_parameter("xp", [128, LPX, P2], BF16, isOutput=False)
    yp_d = nc.declare_dram_parameter("yp", [128, LPY, P2], BF16, isOutput=False)
    wi_d = nc.declare_dram_parameter("wi", [128, 128], BF16, isOutput=False)
    wf_d = nc.declare_dram_parameter("wf", [128, 128], BF16, isOutput=False)
    wm_d = nc.declare_dram_parameter("wm", [82, P2], BF16, isOutput=False)
    bi_d = nc.declare_dram_parameter("bi", [1, NLPA * 128], BF16, isOutput=False)
    bf_d = nc.declare_dram_parameter("bf", [1, NLPA * 128], BF16, isOutput=False)
    out_d = nc.declare_dram_parameter("out", [P2, C, LX], BF16, isOutput=True)

    gath_d = nc.dram_tensor("gath", [NCORES, P2, NBIN, C], BF16,
                            addr_space="Shared")

    MUL = mybir.AluOpType.mult
    MAX = mybir.AluOpType.max
    ADD = mybir.AluOpType.add

    def evict(eng, dst, src, lrelu=False):
        if lrelu:
            assert eng == "a"
            nc.scalar.activation(dst, src, mybir.ActivationFunctionType.Prelu,
                                 alpha=0.2)
        elif eng == "a":
            nc.scalar.copy(dst, src)
        else:
            e = {"v": nc.vector, "g": nc.gpsimd}[eng]
            e.tensor_copy(dst, src)

    with nc.allow_low_precision("bf16 compute pipeline"), TileContext(nc) as tc:
        with (
            tc.tile_pool(name="const", bufs=1) as constp,
            tc.tile_pool(name="stage", bufs=2) as stagep,
            tc.tile_pool(name="feap", bufs=2) as feap,
            tc.tile_pool(name="treep", bufs=1) as treep,
            tc.tile_pool(name="imgp", bufs=2) as imgp,
            tc.tile_pool(name="evp", bufs=3) as evp,
            tc.tile_pool(name="ps", bufs=2, space="PSUM") as psp,
            tc.tile_pool(name="dram", bufs=1, space="DRAM") as dramp,
        ):
            wi_sb = constp.tile([128, 128], BF16, tag="wi")
            wf_sb = constp.tile([128, 128], BF16, tag="wf")
            wm_sb = constp.tile([82, P2], BF16, tag="wm")
            ms0 = constp.tile([82, NLPA, 128], BF16, tag="ms0")
            ms1 = constp.tile([82, NLPA, 128], BF16, tag="ms1")
            h32 = constp.tile([P2, 2, NU, C], BF16, tag="h32")
            b1t = constp.tile([P2, NU, C], BF16, tag="b1t")
            part = constp.tile([P2, NBIN, C], BF16, tag="part")
            pooled = constp.tile([P2, C, P2], BF16, tag="pooled")
            stg = constp.tile([P2, NCORES, NBIN, C], BF16, tag="stg")
            part_dram = dramp.tile([P2, NBIN, C], BF16)

            nc.sync.dma_start(out=wi_sb[:, :], in_=wi_d[:, :])
            nc.sync.dma_start(out=wf_sb[:, :], in_=wf_d[:, :])
            nc.sync.dma_start(out=wm_sb[:, :], in_=wm_d[:, :])
            # conv-bias contraction rows, written once per pass
            nc.sync.dma_start(
                out=ms0[81:82, :, :].rearrange("p a b -> p (a b)"), in_=bf_d[:, :]
            )
            nc.sync.dma_start(
                out=ms1[81:82, :, :].rearrange("p a b -> p (a b)"), in_=bf_d[:, :]
            )

            def conv_chunk(st_src, lp0, nlp, w_sb, ms, engs, att_cb=None):
                """stage DMA + conv matmuls + evicts into ms rows 0..80.
                att_cb(t) is called between conv tiles to interleave
                attention tiles into the PE stream."""
                st = stagep.tile([128, NLPA, P2], BF16, tag="st")
                nc.sync.dma_start(
                    out=st[:, 0:nlp, :], in_=st_src[:, lp0:lp0 + nlp, :]
                )
                for t in range(nlp // 4):
                    cv = psp.tile([P2, 512], F32, tag="cv")
                    for j in range(4):
                        nc.tensor.matmul(
                            cv[0:P2, 128 * j:128 * (j + 1)],
                            st[:, 4 * t + j, :], w_sb[:, :],
                            start=True, stop=True,
                        )
                    d = ms[0:P2, 4 * t:4 * t + 4, :].rearrange("p a b -> p (a b)")
                    evict(engs[t], d, cv[0:P2, :])
                    if att_cb is not None:
                        att_cb(t)

            def mlp_tiles(ms, nlp, dst, dst_lp0, wide):
                """MLP matmuls + ACT Prelu evicts into dst at pair offset.
                wide: 8-pair [81,1024] tiles on tag "at" (pass A / ring 0)
                vs 4-pair [81,512] tiles on tag "ml" (att-active rings)."""
                flat = ms[:, :, :].rearrange("p a b -> p (a b)")
                if wide:
                    for t in range(nlp // 8):
                        mp = psp.tile([P2, 1024], F32, tag="at")
                        for g in range(2):
                            nc.tensor.matmul(
                                mp[0:P2, 512 * g:512 * (g + 1)],
                                wm_sb[:, :],
                                flat[:, 1024 * t + 512 * g:
                                     1024 * t + 512 * (g + 1)],
                                start=True, stop=True,
                            )
                        dq = dst[0:P2, dst_lp0 + 8 * t:dst_lp0 + 8 * t + 8,
                                 :].rearrange("p a b -> p (a b)")
                        evict("a", dq, mp[0:P2, :], lrelu=True)
                else:
                    for t in range(nlp // 4):
                        mp = psp.tile([P2, 512], F32, tag="ml")
                        nc.tensor.matmul(
                            mp[0:P2, 0:512], wm_sb[:, :],
                            flat[:, 512 * t:512 * (t + 1)],
                            start=True, stop=True,
                        )
                        dq = dst[0:P2, dst_lp0 + 4 * t:dst_lp0 + 4 * t + 4,
                                 :].rearrange("p a b -> p (a b)")
                        evict("a", dq, mp[0:P2, :], lrelu=True)

            # ---------------- pass A: fea (y) + local pooled bins -----------
            for ch in range(NCHA):
                ms = ms0 if ch % 2 == 0 else ms1
                # conv evicts: DVE, ACT, ACT, GP; mlp: ACT, GP, GP + 1 ACT
                cengs = (["v", "v", "a", "v", "v", "v", "a", "v"]
                         if ch % 2 == 0 else
                         ["v", "v", "a", "v", "v", "v", "v", "v"])
                conv_chunk(yp_d, NLPA * ch, NLPA, wf_sb, ms, cengs)
                fea = feap.tile([P2, NLPA, 128], BF16, tag="fea")
                mlp_tiles(ms, NLPA, fea, 0, wide=True)
                # pairwise tree over lp: 32 -> 16 -> 8 -> 4 -> 2 -> 1
                # (bf16 2x DVE); 32-patch sums land in h32
                t1 = treep.tile([P2, 16, 128], BF16, tag="t1")
                f2 = fea[0:P2, :, :].rearrange("p (a two) b -> p a two b", two=2)
                nc.vector.tensor_tensor(
                    t1[0:P2, :, :], f2[:, :, 0, :], f2[:, :, 1, :], ADD)
                t2 = treep.tile([P2, 8, 128], BF16, tag="t2")
                t1s = t1[0:P2, :, :].rearrange("p (a two) b -> p a two b", two=2)
                nc.gpsimd.tensor_tensor(
                    t2[0:P2, :, :], t1s[:, :, 0, :], t1s[:, :, 1, :], ADD)
                t3 = treep.tile([P2, 4, 128], BF16, tag="t3")
                t2s = t2[0:P2, :, :].rearrange("p (a two) b -> p a two b", two=2)
                nc.gpsimd.tensor_tensor(
                    t3[0:P2, :, :], t2s[:, :, 0, :], t2s[:, :, 1, :], ADD)
                t4 = treep.tile([P2, 2, 128], BF16, tag="t4")
                t3s = t3[0:P2, :, :].rearrange("p (a two) b -> p a two b", two=2)
                nc.gpsimd.tensor_tensor(
                    t4[0:P2, :, :], t3s[:, :, 0, :], t3s[:, :, 1, :], ADD)
                nc.gpsimd.tensor_tensor(
                    h32[0:P2, :, ch, :],
                    t4[0:P2, 0, :].rearrange("p (s c) -> p s c", s=2),
                    t4[0:P2, 1, :].rearrange("p (s c) -> p s c", s=2),
                    ADD)

            # bin tree: 44 unit sums (t = 22*slot + u are consecutive global
            # 32-patch units) -> 22 -> 11 bins of 128 patches
            hf = h32[0:P2, :, :, :].rearrange("p s u c -> p (s u) c")
            hp = hf.rearrange("p (a two) c -> p a two c", two=2)
            nc.gpsimd.tensor_tensor(
                b1t[0:P2, :, :], hp[:, :, 0, :], hp[:, :, 1, :], ADD)
            b1p = b1t[0:P2, :, :].rearrange("p (a two) c -> p a two c", two=2)
            nc.gpsimd.tensor_tensor(
                part[0:P2, :, :], b1p[:, :, 0, :], b1p[:, :, 1, :], ADD)
            nc.gpsimd.dma_start(out=part_dram[:, :, :], in_=part[:, :, :])
            nc.gpsimd.collective_compute(
                "AllGather",
                mybir.AluOpType.bypass,
                replica_groups=[list(range(NCORES))],
                ins=[part_dram[:, :, :]],
                outs=[gath_d[:, :, :, :]],
            )

            # rewrite bias rows for pass B
            nc.sync.dma_start(
                out=ms0[81:82, :, :].rearrange("p a b -> p (a b)"), in_=bi_d[:, :]
            )
            nc.sync.dma_start(
                out=ms1[81:82, :, :].rearrange("p a b -> p (a b)"), in_=bi_d[:, :]
            )

            # ---------------- pass B: img (x) + attention -------------------
            def att_tiles(img, r_att, i0, n, ev_engs, dma_engs):
                lp0 = RING * r_att
                rhsv = img[0:P2, :, :].rearrange("p l (s c) -> p c s l", s=2)
                for i in range(i0, i0 + n):
                    ap = psp.tile([P2, 1024], F32, tag="at")
                    for g in range(2):
                        c = 2 * i + g
                        nc.tensor.matmul(
                            ap[0:P2, 512 * g:512 * g + 2 * RING],
                            pooled[:, c:c + 1, :], rhsv[:, c:c + 1, :, :],
                            start=True, stop=True,
                        )
                    ev = evp.tile([P2, 2, 2, RING], BF16, tag="ev")
                    src = ap[0:P2, :].rearrange(
                        "p (g x) -> p g x", g=2)[:, :, 0:2 * RING].rearrange(
                        "p g (s l) -> p g s l", s=2)
                    evict(ev_engs[i - i0], ev[0:P2, :, :, :], src)
                    dst = out_d[0:P2, 2 * i:2 * i + 2, :].rearrange(
                        "p c (s l) -> p c s l", s=2)[:, :, :, lp0:lp0 + RING]
                    if dma_engs[i - i0] == "g":
                        nc.gpsimd.dma_start(out=dst, in_=ev[0:P2, :, :, :])
                    else:
                        nc.sync.dma_start(out=dst, in_=ev[0:P2, :, :, :])

            imgs = []
            msc = 0
            for r in range(NRING):
                img = imgp.tile([P2, RING, 128], BF16, tag="img")
                imgs.append(img)
                for chb in range(NCHB):
                    ms = ms0 if msc % 2 == 0 else ms1
                    msc += 1
                    att_cb = None
                    if r >= 1 and chb < 8:
                        def att_cb(t, _img=imgs[r - 1], _r=r - 1, _chb=chb):
                            if t in (1, 2, 3, 4):
                                att_tiles(_img, _r, 4 * _chb + t - 1, 1,
                                          ["v" if t < 3 else "a"],
                                          ["s" if t % 2 else "g"])
                    cengs = ["v"] * 6
                    conv_chunk(xp_d, RING * r + NLPB * chb, NLPB, wi_sb, ms,
                               cengs, att_cb)
                    mlp_tiles(ms, NLPB, img, NLPB * chb, wide=(r == 0))
                if r == 0:
                    # pooled assembly (collective long since done)
                    nc.gpsimd.dma_start(
                        out=stg[:, :, :, :],
                        in_=gath_d[:, :, :, :].rearrange("k p b c -> p k b c"),
                    )
                    nc.vector.tensor_scalar_mul(
                        pooled[0:P2, :, 0:80].rearrange(
                            "p c (k b) -> p k b c", k=8),
                        stg[0:P2, :, 0:10, :], 1.0 / 128.0,
                    )
                    nc.vector.tensor_scalar_mul(
                        pooled[0:P2, :, 80:81].rearrange("p c j -> p j c"),
                        stg[0:P2, 7:8, 10:11, :].rearrange(
                            "p k b c -> p (k b) c"),
                        1.0 / 128.0,
                    )
            # trailing attention for the last ring
            ev_engs = [["v", "a"][i % 2] for i in range(NATT)]
            dma_engs = ["g" if i % 4 == 1 else "s" for i in range(NATT)]
            att_tiles(imgs[2], 2, 0, NATT, ev_engs, dma_engs)
    nc.compile()
    return nc


def _host_prep(x, y, w_img, b_img, w_fea, b_fea, w1, w2):
    f32 = np.float32
    bf16 = ml_dtypes.bfloat16
    weff = (w2.astype(np.float64) @ w1.astype(np.float64))  # (81, 81)
    wm = np.concatenate([weff.T, weff.sum(axis=1)[None, :]], axis=0)
    wm = wm.astype(f32).astype(bf16)

    def pairw(w):
        blk = np.zeros((128, 128), dtype=f32)
        blk[0:64, 0:64] = w.T
        blk[64:128, 64:128] = w.T
        return blk.astype(bf16)

    wi = pairw(w_img.astype(f32))
    wf = pairw(w_fea.astype(f32))
    bi = np.tile(np.concatenate([b_img, b_img]).astype(f32), NLPA)[None, :]
    bf_ = np.tile(np.concatenate([b_fea, b_fea]).astype(f32), NLPA)[None, :]
    bi = bi.astype(bf16)
    bf_ = bf_.astype(bf16)

    def unfold(t):  # (1, 64, 72, 108, 108) -> (C, 10368, 81) patch matrix
        u = np.ascontiguousarray(
            t.reshape(C, D // P, P, HW // P, P).transpose(0, 1, 3, 2, 4)
        ).reshape(C, (D // P) * (HW // P), P2)
        return u

    def pack(u, l0, lhalf):  # global patches [l0, l0+2*lhalf) -> [128, lhalf, 81]
        v = u[:, l0:l0 + 2 * lhalf, :].reshape(C, 2, lhalf, P2)
        v = v.transpose(1, 0, 2, 3).reshape(128, lhalf, P2)
        return np.ascontiguousarray(v).astype(ml_dtypes.bfloat16)

    ux = unfold(np.asarray(x, dtype=f32))
    uy = unfold(np.asarray(y, dtype=f32))
    shared = {"wi": wi, "wf": wf, "wm": wm, "bi": bi, "bf": bf_}
    maps = []
    for k in range(NCORES):
        maps.append(dict(
            shared,
            xp=pack(ux, LX * k, LPX),
            yp=pack(uy, (LY - 128) * k, LPY),
        ))
    return maps


def kernel(x, y, w_img, b_img, w_fea, b_fea, w1, w2):
    if "nc" not in _cache:
        _cache["nc"] = _build_nc()
    nc = _cache["nc"]
    in_maps = _host_prep(x, y, w_img, b_img, w_fea, b_fea, w1, w2)
    trace = bool(os.environ.get("KERNEL_TRACE"))
    res = run_bass_kernel_spmd(
        nc, in_maps, list(range(NCORES)), trace=trace
    )
    _cache["last_result"] = res
    out = np.empty((1, C, D, H, W), dtype=np.float32)
    ov = out.reshape(C, D, HW)
    for k in range(NCORES):
        # out_d is (81, 64, 1296) with l = 648*slot + lp (already global l)
        att = res.results[k]["out"].astype(np.float32).transpose(1, 2, 0)
        blk = att.reshape(C, LX, P, P).transpose(0, 2, 1, 3).reshape(C, P, HW)
        ov[:, P * k:P * (k + 1), :] = blk
    return out
